# revision 35
# baseline (speedup 1.0000x reference)
"""MixHop layer (powers 0,1,2) Trainium2 Bass kernel.

Problem (per batch b, 8 batches, one NeuronCore each):
    h_p = x_b @ W_p          (x: [F=64, N=2048, T=12], W: [64, 64])
    g_p = adj_b^p @ h_p      (adj: [N, N], diffusion applied p times)
    out_p = leaky_relu(g_p, 0.01)
    out = concat([out_0, out_1, out_2], channel axis) -> [B, 192, N, T]

Design notes (v3):
  - Data-parallel over batch: core b handles batch b.
  - bf16 operands for phase 1 and the adj diffusion (PE streams 1 col/cycle
    either way; psum accumulates fp32), giving l2rel ~3e-3 (gate is 2e-2).
  - Phase 1 (h = x@W): x chunks stationary, rhs wz [128, 384] = [W1|W2|W0]
    for two t-planes block-diagonally. Psum slots hold a th-pair; one
    copy per slot (DVE for 2 of 3 slots, ACT for the third) drains to hall.
  - Pass A streams adjT slabs for w = adj@h2; out0 = lrelu(hall.h0) is
    drained in bulk during pass A.
  - z1.T = h1.T @ adjT in fp8e4 DoubleRow (K=256/pass, 2 fp8 elem/cycle
    with pair-interleaved moving layout). Power-1's error contribution is
    divided by ~680x in the combined l2 norm, so fp8 is numerically safe.
  - Pass B streams adjT again for z2 = adj@w.
  - Loads ride nc.sync; stores ride nc.scalar (both HWDGE). gpsimd/SWDGE is
    never used - its descriptor generation contends with DVE for SBUF.
"""

import os
import sys

if "/opt/trn_rl_repo" not in sys.path:
    sys.path.insert(0, "/opt/trn_rl_repo")

import numpy as np
import ml_dtypes

import concourse.bass as bass
import concourse.tile as tile
from concourse import bacc, mybir
from concourse.bass_utils import run_bass_kernel_spmd

F = 64          # input features
O = 64          # output features per power
N = 2048        # nodes
T = 12          # time steps
NB = N // 128   # 16 node blocks
NT = N * T      # 24576
C = O * T       # 768 columns per power, (t, o) ordering

F32 = mybir.dt.float32
BF16 = mybir.dt.bfloat16
FP8 = mybir.dt.float8e4

Z1_FP8 = os.environ.get("Z1_FP8", "1") == "1"


def build_nc():
    nc = bacc.Bacc("TRN2", target_bir_lowering=False, debug=False, num_devices=8)

    # ---- DRAM I/O ----------------------------------------------------------
    # x2: [(tl, f) = 128, (mb, th, nl) = 12288] where t = 2*th + tl.
    x_d = nc.dram_tensor("x", [128, NT // 2], BF16, kind="ExternalInput").ap()
    # adjT tiled: [nb, p, mb, nl] where adjT[m, n] = adj[n, m], m = mb*128+p,
    # n = nb*128+nl. One [p, (mb nl)] slab per nb is a contiguous 512 KiB read.
    adjt_d = nc.dram_tensor("adjt", [NB, 128, NB, 128], BF16, kind="ExternalInput").ap()
    # wz: [128, 256] = block-diag over the two t-planes; each 128-block is
    # [W1 | W2]. wz0: [128, 128] block-diag [W0 | W0] for the out0 pass.
    wz_d = nc.dram_tensor("wz", [128, 256], BF16, kind="ExternalInput").ap()
    wz0_d = nc.dram_tensor("wz0", [128, 128], BF16, kind="ExternalInput").ap()
    if Z1_FP8:
        # adj8: [p, j, n, i] fp8e4 with adj8[p,j,n,i] = adjT[(2j+i)*128+p, n];
        # the K-pair partner elements are adjacent (i innermost) so DoubleRow
        # streams 2 fp8/cycle.
        adj8_d = nc.dram_tensor("adj8", [128, 8, N, 2], FP8, kind="ExternalInput").ap()

    # out0t: [(tl, o) = 128, (mb, th, nl) = 12288] = lrelu(x@W0) transposed;
    # the host untransposes during unshard.
    out0_d = nc.dram_tensor("out0t", [128, NT // 2], F32, kind="ExternalOutput").ap()
    if Z1_FP8:
        z1_d = nc.dram_tensor("z1t", [C, N], F32, kind="ExternalOutput").ap()
    else:
        z1_d = nc.dram_tensor("z1", [N, C], F32, kind="ExternalOutput").ap()
    z2_d = nc.dram_tensor("z2", [N, C], F32, kind="ExternalOutput").ap()

    lrelu = mybir.ActivationFunctionType.Lrelu
    act_copy = mybir.ActivationFunctionType.Copy

    with tile.TileContext(nc) as tc:
        with (
            tc.tile_pool(name="consts", bufs=1) as consts,
            tc.tile_pool(name="xres", bufs=8) as xres,
            tc.tile_pool(name="hall", bufs=NB) as hallp,
            tc.tile_pool(name="wbuf", bufs=NB) as wbufp,
            tc.tile_pool(name="adjt", bufs=3) as adjp,
            tc.tile_pool(name="zst", bufs=3) as zstp,
            tc.tile_pool(name="o0st", bufs=3) as o0p,
        ):
            # consts + resident x (8 tiles); all loads on the sync ring
            wz_t = consts.tile([128, 256], BF16)
            nc.sync.dma_start(out=wz_t[:], in_=wz_d)
            wz0_t = consts.tile([128, 128], BF16)
            nc.sync.dma_start(out=wz0_t[:], in_=wz0_d)
            slabs = {}

            def load_slab(i):
                s = adjp.tile([128, N], BF16, tag="slab")
                nc.sync.dma_start(
                    out=s[:], in_=adjt_d[i % NB].rearrange("p a b -> p (a b)")
                )
                slabs[i] = s

            xt = []
            for q in range(8):
                xq = xres.tile([128, 1536], BF16, tag="x", name=f"x{q}")
                nc.sync.dma_start(out=xq[:], in_=x_d[:, q * 1536 : (q + 1) * 1536])
                xt.append(xq)
                if q == 1:
                    load_slab(0)  # slab0 early: phase 1 interleaves pass A nb=0
            for i in range(1, 3):
                load_slab(i)

            if Z1_FP8:
                adj8_t = consts.tile([128, 8 * N * 2], FP8)
                h18_t = consts.tile([128, NB * C], FP8)
                adj8_flat = adj8_d.rearrange("p a b c -> p (a b c)")

            # ---- Phase 1: h12 = x @ [W1 W2] --------------------------------
            # One [128, 1536] psum tile per mb (6 th-MMs of 256 cols each, all
            # 256-aligned so none cross a psum bank); a single contiguous
            # 1536-col drain per mb alternates between DVE and ACT. Pass A's
            # nb=0 accumulation rides along (one mb behind) so the PE stays
            # dense and the HAM clock-gate stays open.
            hall = []
            wtiles = []
            head = Z1_FP8
            with (
                tc.tile_pool(name="ps_ph1", bufs=2, space="PSUM") as ps1,
                tc.tile_pool(name="ps_head", bufs=1, space="PSUM") as psh,
            ):
                if head:
                    wph = psh.tile([128, 1024], F32, tag="headw")
                for mb in range(NB):
                    xm = xt[mb // 2][:, (mb % 2) * 768 : (mb % 2 + 1) * 768]
                    hall_t = hallp.tile([128, 2 * C], BF16, tag="hall")
                    hall.append(hall_t)
                    ph = ps1.tile([128, 1536], F32, tag="ph1")
                    for th in range(6):
                        nc.tensor.matmul(
                            ph[:, th * 256 : (th + 1) * 256],
                            xm[:, th * 128 : (th + 1) * 128],
                            wz_t[:],
                            start=True,
                            stop=True,
                        )
                    if mb % 2 == 0:
                        nc.vector.tensor_copy(hall_t[:], ph[:])
                    else:
                        nc.scalar.activation(hall_t[:], ph[:], act_copy)
                    if head:
                        lhsT = slabs[0][:, mb * 128 : (mb + 1) * 128]
                        st, sp = mb == 0, mb == NB - 1
                        nc.tensor.matmul(
                            wph[:, 0:512], lhsT,
                            hall_t[:].rearrange("p (t z o) -> p t z o", t=T, z=2)[
                                :, 0:8, 1, :
                            ],
                            start=st, stop=sp,
                        )
                        nc.tensor.matmul(
                            wph[:, 512:768], lhsT,
                            hall_t[:].rearrange("p (t z o) -> p t z o", t=T, z=2)[
                                :, 8:12, 1, :
                            ],
                            start=st, stop=sp,
                        )
                if head:
                    w_t = wbufp.tile([128, C], BF16, tag="w")
                    wtiles.append(w_t)
                    nc.vector.tensor_copy(w_t[:], wph[:, 0:C])
                    slabs.pop(0)

            # hall views: cols = (t, z, o), z in {0: W1(h1), 1: W2(h2)}
            def hview(mb, z, t0, t1):
                return hall[mb][:].rearrange(
                    "p (t z o) -> p t z o", t=T, z=2
                )[:, t0:t1, z, :]

            # ---- Pass A: stream adjT; w = adj@h2 (and z1 = adj@h1 if bf16).
            # Interleaved: out0t = lrelu(wz0.T @ x) in 24 512-col MM chunks
            # (2 per nb for the first 12 nb), drained by ACT which is
            # otherwise idle here. The psum pool is shared with z1T and
            # pass B so no pool-transition barrier splits the phases.
            with (
                tc.tile_pool(name="ps_pa", bufs=3, space="PSUM") as psa,
                tc.tile_pool(name="ps_o0", bufs=2, space="PSUM") as pso,
                tc.tile_pool(name="z1tst", bufs=3) as z1tp,
            ):
                nsl = [3]

                def prefetch_slab():
                    if nsl[0] < 2 * NB:
                        load_slab(nsl[0])
                        nsl[0] += 1

                for nb in range(1 if Z1_FP8 else 0, NB):
                    if Z1_FP8 and 1 <= nb <= 4:
                        # adj8 in 4 x 1 MiB chunks: short transfers never
                        # block the DMA-completion sem lanes for long
                        q = nb - 1
                        nc.sync.dma_start(
                            out=adj8_t[:, q * 8192 : (q + 1) * 8192],
                            in_=adj8_flat[:, q * 8192 : (q + 1) * 8192],
                        )
                    base = 2 * (nb - 1) if Z1_FP8 else 2 * nb
                    for c in range(base, base + 2):
                        if c >= 24:
                            continue
                        po = pso.tile([128, 512], F32, tag="o0ps")
                        nc.tensor.matmul(
                            po[:],
                            wz0_t[:],
                            xt[c // 3][:, (c % 3) * 512 : (c % 3 + 1) * 512],
                            start=True,
                            stop=True,
                        )
                        o0 = o0p.tile([128, 512], F32, tag="o0")
                        nc.scalar.activation(o0[:], po[:], lrelu, alpha=0.01)
                        nc.scalar.dma_start(
                            out=out0_d[:, c * 512 : (c + 1) * 512], in_=o0[:]
                        )
                    if Z1_FP8 and 8 <= nb <= 15:
                        # h18 = fp8(h1), emitted late so the scheduler keeps
                        # DVE free for the phase-1 psum drains
                        for mb in (2 * (nb - 8), 2 * (nb - 8) + 1):
                            nc.vector.tensor_copy(
                                h18_t[:, mb * C : (mb + 1) * C].rearrange(
                                    "p (t o) -> p t o", t=T
                                ),
                                hview(mb, 0, 0, T),
                            )
                    prefetch_slab()
                    slab = slabs.pop(nb)
                    wp = psa.tile([128, 1024], F32, tag="big")
                    if not Z1_FP8:
                        z1p = psa.tile([128, 1024], F32, tag="big")
                    for mb in range(NB):
                        lhsT = slab[:, mb * 128 : (mb + 1) * 128]
                        st, sp = mb == 0, mb == NB - 1
                        nc.tensor.matmul(
                            wp[:, 0:512], lhsT, hview(mb, 1, 0, 8), start=st, stop=sp
                        )
                        nc.tensor.matmul(
                            wp[:, 512:768], lhsT, hview(mb, 1, 8, 12), start=st, stop=sp
                        )
                        if not Z1_FP8:
                            nc.tensor.matmul(
                                z1p[:, 0:512], lhsT, hview(mb, 0, 0, 8),
                                start=st, stop=sp,
                            )
                            nc.tensor.matmul(
                                z1p[:, 512:768], lhsT, hview(mb, 0, 8, 12),
                                start=st, stop=sp,
                            )
                    # drains
                    w_t = wbufp.tile([128, C], BF16, tag="w")
                    wtiles.append(w_t)
                    nc.vector.tensor_copy(w_t[:], wp[:, 0:C])
                    if not Z1_FP8:
                        zt = zstp.tile([128, C], F32, tag="zst")
                        nc.scalar.activation(zt[:], z1p[:, 0:C], lrelu, alpha=0.01)
                        nc.scalar.dma_start(
                            out=z1_d[nb * 128 : (nb + 1) * 128, :], in_=zt[:]
                        )


                # ---- z1.T phase (fp8 DoubleRow): z1.T = h1.T @ adjT --------
                if Z1_FP8:
                    adj8v = adj8_t[:].rearrange("p (j n i) -> p j n i", j=8, i=2)
                    h18v = h18_t[:].rearrange("p (mb c) -> p mb c", mb=NB)
                    for cb in range(6):
                        for half in range(2):
                            n0 = half * 1024
                            zp = psa.tile([128, 1024], F32, tag="big")
                            for j in range(8):
                                lhsT = h18v[
                                    :, 2 * j : 2 * j + 2, cb * 128 : (cb + 1) * 128
                                ]
                                for q in range(2):
                                    rhs = adj8v[
                                        :, j, n0 + q * 512 : n0 + (q + 1) * 512, :
                                    ].rearrange("p n i -> p i n")
                                    nc.tensor.matmul(
                                        zp[:, q * 512 : (q + 1) * 512],
                                        lhsT,
                                        rhs,
                                        start=(j == 0),
                                        stop=(j == 7),
                                        perf_mode=mybir.MatmulPerfMode.DoubleRow,
                                    )
                            zt = z1tp.tile([128, 1024], F32, tag="z1tst")
                            nc.scalar.activation(zt[:], zp[:], lrelu, alpha=0.01)
                            nc.scalar.dma_start(
                                out=z1_d[
                                    cb * 128 : (cb + 1) * 128, n0 : n0 + 1024
                                ],
                                in_=zt[:],
                            )

                # ---- Pass B: stream adjT again; z2 = adj@w -----------------
                for nb in range(NB):
                    i = NB + nb
                    prefetch_slab()
                    slab = slabs.pop(i)
                    pz = psa.tile([128, 1024], F32, tag="big")
                    for mb in range(NB):
                        lhsT = slab[:, mb * 128 : (mb + 1) * 128]
                        st, sp = mb == 0, mb == NB - 1
                        nc.tensor.matmul(
                            pz[:, 0:512], lhsT, wtiles[mb][:, 0:512], start=st, stop=sp
                        )
                        nc.tensor.matmul(
                            pz[:, 512:768], lhsT, wtiles[mb][:, 512:768],
                            start=st, stop=sp,
                        )
                    if nb < NB - 1:
                        zt = zstp.tile([128, C], F32, tag="zst")
                        nc.scalar.activation(zt[:], pz[:, 0:C], lrelu, alpha=0.01)
                        nc.scalar.dma_start(
                            out=z2_d[nb * 128 : (nb + 1) * 128, :], in_=zt[:]
                        )
                    else:
                        # split the final drain so ACT/store pipeline the tail
                        for hh in range(4):
                            zt = zstp.tile([128, C // 4], F32, tag="zsth")
                            nc.scalar.activation(
                                zt[:], pz[:, hh * 192 : (hh + 1) * 192],
                                lrelu, alpha=0.01,
                            )
                            nc.scalar.dma_start(
                                out=z2_d[
                                    nb * 128 : (nb + 1) * 128,
                                    hh * 192 : (hh + 1) * 192,
                                ],
                                in_=zt[:],
                            )

    nc.finalize()
    return nc


_NC = None
LAST_RESULTS = None  # stashed BassKernelResults for test harnesses


def kernel(x, adj, W0, b0, W1, b1, W2, b2):
    """Full inputs in, full output out. Shards batch b -> core b."""
    global _NC, LAST_RESULTS
    x = np.asarray(x, dtype=np.float32)
    adj = np.asarray(adj, dtype=np.float32)
    W0 = np.asarray(W0, dtype=np.float32)
    W1 = np.asarray(W1, dtype=np.float32)
    W2 = np.asarray(W2, dtype=np.float32)
    B = x.shape[0]
    assert B == 8 and x.shape == (8, F, N, T) and adj.shape == (8, N, N)

    if _NC is None:
        _NC = build_nc()

    bf16 = ml_dtypes.bfloat16
    # x: [B, F, N, T] -> [B, (tl, f) = 128, (mb, th, nl)], t = 2*th + tl
    xr = np.ascontiguousarray(
        x.reshape(B, F, NB, 128, T // 2, 2).transpose(0, 5, 1, 2, 4, 3)
    ).reshape(B, 128, NT // 2).astype(bf16)
    # adjT tiled: [B, nb, p, mb, nl];  adjT[m, n] = adj[n, m]
    adjT = np.ascontiguousarray(adj.transpose(0, 2, 1))  # [B, m, n]
    adjt = np.ascontiguousarray(
        adjT.reshape(B, NB, 128, NB, 128).transpose(0, 3, 2, 1, 4)
    ).astype(bf16)
    # wz: [128, 256] block-diag, each 128-block = [W1 | W2]; wz0 likewise W0
    wcat = np.concatenate([W1, W2], axis=1)  # [64, 128]
    wz = np.zeros((128, 256), dtype=np.float32)
    wz[0:F, 0:128] = wcat
    wz[F:128, 128:256] = wcat
    wz = wz.astype(bf16)
    wz0 = np.zeros((128, 128), dtype=np.float32)
    wz0[0:F, 0:O] = W0
    wz0[F:128, O:128] = W0
    wz0 = wz0.astype(bf16)

    in_maps = []
    for b in range(B):
        m = {"x": xr[b], "adjt": adjt[b], "wz": wz, "wz0": wz0}
        if Z1_FP8:
            # adj8: [p, j, n, i] = adjT[(2j+i)*128+p, n], pair-interleaved
            m["adj8"] = np.ascontiguousarray(
                adjT[b].reshape(8, 2, 128, N).transpose(2, 0, 3, 1)
            ).astype(ml_dtypes.float8_e4m3)
        in_maps.append(m)

    nwarm = int(os.environ.get("KERNEL_WARMUP_RUNS", "0"))
    for _ in range(nwarm):
        run_bass_kernel_spmd(_NC, in_maps, core_ids=list(range(8)))
    res = run_bass_kernel_spmd(_NC, in_maps, core_ids=list(range(8)))
    LAST_RESULTS = res

    out = np.empty((B, 3 * O, N, T), dtype=np.float32)
    for b in range(B):
        r = res.results[b]
        # out0t: [(tl, o), (mb, th, nl)] -> [o, n, t], t = 2*th + tl
        out[b, 0:O] = (
            r["out0t"]
            .reshape(2, O, NB, T // 2, 128)
            .transpose(1, 2, 4, 3, 0)
            .reshape(O, N, T)
        )
        if Z1_FP8:
            out[b, O : 2 * O] = r["z1t"].reshape(T, O, N).transpose(1, 2, 0)
        else:
            out[b, O : 2 * O] = r["z1"].reshape(N, T, O).transpose(2, 0, 1)
        out[b, 2 * O : 3 * O] = r["z2"].reshape(N, T, O).transpose(2, 0, 1)
    del b0, b1, b2
    return out
